# revision 20
# baseline (speedup 1.0000x reference)
"""Bass/Trainium2 kernel for nn_BellmanLoss (8-core data-parallel).

Math: the reference's scatter makes Q_new differ from Q0 only at
a_i = argmax_j(actions[i, j]) (first max), so

    loss = sum_i (Q0[i, a_i] - target_i)^2
    target_i = r_i + 0.9 * max_a Qn[i, a] * (1 - done_i),   done_i = (states1[i,0] == 666)

Per core: 8192 rows. MLP runs feature-major (h^T = [features, batch]) so the
weights are the stationary matmul operand. Layer 3 is packed 4 ticks at a
time into one PSUM tile via 32-wide col groups (tile_position), so the
narrow (A=18) output stops wasting full-width PE passes; the packed tile is
PE-transposed to batch-major in [128,128] blocks and a fused vector epilogue
computes the argmax-select, max_a, target and per-partition loss partials.
Host does layout-only prep (transpose/reshape/cast, group permutation) and
the final 1024-element sum.
"""

import os
import numpy as np

import concourse.bass as bass
import concourse.mybir as mybir
import concourse.tile as tile
from concourse import bacc
from concourse.bass_utils import run_bass_kernel_spmd

# Problem constants (hardcoded per contract)
B, S, H, A = 65536, 128, 256, 18
NCORES = 8
BC = B // NCORES          # 8192 rows per core
CH = 256                  # batch columns per compute chunk
NCH = BC // CH            # 32 chunks
GR = BC // 128            # 64 groups of 128 rows per core
LOADCOLS = 1024           # x DMA tile columns
NLOAD = BC // LOADCOLS
QW = 32                   # padded action dim per col group
QOFF = 32                 # Qn offset within a 64-col group slot
NPACK = 16                # mm3 packs (4 ticks each)
DONE = 666.0
DISC = 0.9

MM_DT = os.environ.get("BELLMAN_MM_DT", "bf16")  # "bf16" | "f32r"
MM2 = os.environ.get("BELLMAN_MM2", "bf16")  # "bf16" | "dr" | "drsw"
EP_LIMIT = int(os.environ.get("BELLMAN_EP", "9999"))

if MM_DT == "bf16":
    import ml_dtypes
    NP_MDT = ml_dtypes.bfloat16
    TILE_MDT = mybir.dt.bfloat16
else:
    NP_MDT = np.float32
    TILE_MDT = mybir.dt.float32r

F32 = mybir.dt.float32
I32 = mybir.dt.int32
FP8 = mybir.dt.float8e4
import ml_dtypes as _mld
NP_FP8 = _mld.float8_e4m3
H1_DT = FP8 if MM2 in ("dr", "drsw") else TILE_MDT
NP_H1 = NP_FP8 if MM2 in ("dr", "drsw") else NP_MDT
AF = mybir.ActivationFunctionType
OP = mybir.AluOpType
AX = mybir.AxisListType


def _build_program():
    nc = bacc.Bacc("TRN2", target_bir_lowering=False, debug=False)

    x0t = nc.dram_tensor("x0t", [128, BC], TILE_MDT, kind="ExternalInput").ap()
    x1t = nc.dram_tensor("x1t", [128, BC], TILE_MDT, kind="ExternalInput").ap()
    actb = nc.dram_tensor("actb", [128, GR * A], I32, kind="ExternalInput").ap()
    rewb = nc.dram_tensor("rewb", [128, GR], F32, kind="ExternalInput").ap()
    s1b = nc.dram_tensor("s1b", [128, GR], F32, kind="ExternalInput").ap()
    w1 = nc.dram_tensor("w1", [S, H], TILE_MDT, kind="ExternalInput").ap()
    if MM2 in ("dr", "drsw"):
        w2 = nc.dram_tensor("w2", [128, 2, 2, 128], FP8, kind="ExternalInput").ap()
    else:
        w2 = nc.dram_tensor("w2", [H, H], TILE_MDT, kind="ExternalInput").ap()
    w3p = nc.dram_tensor("w3p", [H, QW], TILE_MDT, kind="ExternalInput").ap()
    b1d = nc.dram_tensor("b1d", [128, 2], F32, kind="ExternalInput").ap()
    b2d = nc.dram_tensor("b2d", [128, 2], F32, kind="ExternalInput").ap()
    b3d = nc.dram_tensor("b3d", [128, 1], F32, kind="ExternalInput").ap()
    iotad = nc.dram_tensor("iotad", [128, A], F32, kind="ExternalInput").ap()
    identd = nc.dram_tensor("identd", [128, 128], TILE_MDT, kind="ExternalInput").ap()
    outp = nc.dram_tensor("outp", [128, 1], F32, kind="ExternalOutput").ap()

    from contextlib import ExitStack

    with tile.TileContext(nc) as tc, ExitStack() as ctx:
        singles = ctx.enter_context(tc.tile_pool(name="singles", bufs=1))
        xpool = ctx.enter_context(tc.tile_pool(name="xpool", bufs=2))
        hpool = ctx.enter_context(tc.tile_pool(name="hpool", bufs=2))
        qspool = ctx.enter_context(tc.tile_pool(name="qspool", bufs=2))
        big = ctx.enter_context(tc.tile_pool(name="big", bufs=1))
        ps_h1 = ctx.enter_context(tc.tile_pool(name="ps_h1", bufs=3, space="PSUM"))
        ps_h2 = ctx.enter_context(tc.tile_pool(name="ps_h2", bufs=2, space="PSUM"))
        ps_qt = ctx.enter_context(tc.tile_pool(name="ps_qt", bufs=1, space="PSUM"))
        ps_tp = ctx.enter_context(tc.tile_pool(name="ps_tp", bufs=2, space="PSUM"))

        # --- PE warmup: scratch matmuls during the initial DMA wait flip the
        # HAM clock gate to 8/8 before real matmuls arrive ---
        warm_w = singles.tile([128, 128], TILE_MDT, tag="warm_w")
        nc.vector.memset(warm_w, 0.0)
        warm_act = singles.tile([128, 8], F32, tag="warm_act")
        nc.scalar.activation(warm_act, warm_w[:, 0:8], AF.Relu, scale=1.0)
        warm_ps = ps_qt.tile([128, CH], F32, tag="qt", name="warm_ps")
        for i in range(16):
            nc.tensor.matmul(warm_ps[:, 0:128], warm_w, warm_w,
                             start=True, stop=True)

        # --- constants / per-core staging loads (gpsimd queue: keeps the
        # scalar queue free so the first relu copies are not stuck behind
        # slow DMA-issue ops) ---
        w1_s = []
        for m in range(2):
            t = singles.tile([128, 128], TILE_MDT, tag=f"w1_{m}")
            nc.gpsimd.dma_start(out=t, in_=w1[:, m * 128:(m + 1) * 128])
            w1_s.append(t)
        b1_s = singles.tile([128, 2], F32, tag="b1")
        nc.gpsimd.dma_start(out=b1_s, in_=b1d)
        b2_s = singles.tile([128, 2], F32, tag="b2")
        nc.gpsimd.dma_start(out=b2_s, in_=b2d)
        if MM2 in ("dr", "drsw"):
            w2_s = []
            for m in range(2):
                t = singles.tile([128, 2, 128], FP8, tag=f"w2dr_{m}")
                nc.gpsimd.dma_start(out=t, in_=w2[:, m, :, :])
                w2_s.append(t)
        else:
            w2_s = []
            for k in range(2):
                row = []
                for m in range(2):
                    t = singles.tile([128, 128], TILE_MDT, tag=f"w2_{k}{m}")
                    nc.gpsimd.dma_start(
                        out=t, in_=w2[k * 128:(k + 1) * 128, m * 128:(m + 1) * 128])
                    row.append(t)
                w2_s.append(row)
        w3_s = []
        for k in range(2):
            t = singles.tile([128, QW], TILE_MDT, tag=f"w3_{k}", name=f"w3s_{k}")
            w3_s.append(t)
        b3_s = singles.tile([128, 1], F32, tag="b3")
        ident = singles.tile([128, 128], TILE_MDT, tag="ident")
        iota_s = singles.tile([128, A], F32, tag="iota")

        def stage_rest():
            # issued on sync after the first x pieces; none needed before ~17us
            for k in range(2):
                nc.sync.dma_start(out=w3_s[k], in_=w3p[k * 128:(k + 1) * 128, :])
            nc.sync.dma_start(out=b3_s, in_=b3d)
            nc.sync.dma_start(out=ident, in_=identd)
            nc.sync.dma_start(out=iota_s, in_=iotad)

        actb_s = singles.tile([128, GR * A], I32, tag="actb")
        rewb_s = singles.tile([128, GR], F32, tag="rewb")
        s1b_s = singles.tile([128, GR], F32, tag="s1b")

        # batch-major Q staging: slot L (one 128-row slab, host-permuted order)
        # occupies cols [64L, 64L+64): Q at +0..17 (states0), +32..49 (states1).
        qbuf = big.tile([128, GR * 64], TILE_MDT, tag="qbuf")

        relu_idx = [0]

        def relu_copy(dst, src, bias_ap):
            # split relu copies ~17:15 ACT:DVE to equalize engine time
            i = relu_idx[0] % 32
            relu_idx[0] += 1
            if i % 2 == 0 or i == 1:
                nc.scalar.activation(dst, src, AF.Relu, bias=bias_ap, scale=1.0)
            else:
                nc.vector.tensor_scalar(dst, src, bias_ap, 0.0, OP.add, OP.max)

        # ---- software-pipelined main loop ----
        # tick = one chunk-pass (64 ticks). Stage shifts keep every engine's
        # in-order queue free of waits on freshly produced cross-engine data:
        #   t:   mm1[t]          (PE)
        #   t+1: relu1[t]        (ACT/DVE)
        #   t+2: mm2[t]          (PE)
        #   t+3: relu2[t]        (ACT/DVE)
        #   pack p = ticks 4p..4p+3 (2 chunks x 2 states):
        #   4p+7: mm3 pack (PE, 4 col groups), 4p+8: stack copy,
        #   4p+9: transposes, 4p+10: qbuf copy
        T = 2 * NCH
        PASS_PER_LOAD = 2 * LOADCOLS // CH
        xL_tiles = {}
        h1p_t, h1s_t, h2p_t, h2s_t = {}, {}, {}, {}
        qt_p, qts_p, tp_p = {}, {}, {}

        # epilogue tiles (allocated up front; ops emitted inline)
        import itertools
        _ep_count = itertools.count(1)

        def _ep():
            return next(_ep_count) <= EP_LIMIT

        actf = big.tile([128, GR * A], F32, tag="actf")
        score = big.tile([128, GR * A], F32, tag="score")
        rowmax = big.tile([128, GR], F32, tag="rowmax")
        onehot = big.tile([128, GR * A], F32, tag="onehot")
        prod = big.tile([128, GR * A], F32, tag="prod")
        q0sel = big.tile([128, GR], F32, tag="q0sel")
        maxqn = big.tile([128, GR], F32, tag="maxqn")
        donem = big.tile([128, GR], F32, tag="donem")
        fac = big.tile([128, GR], F32, tag="fac")
        t1 = big.tile([128, GR], F32, tag="t1")
        t2 = big.tile([128, GR], F32, tag="t2")
        diff = big.tile([128, GR], F32, tag="diff")
        sq = big.tile([128, GR], F32, tag="sq")
        acc = big.tile([128, 1], F32, tag="acc")
        if EP_LIMIT < 9999:
            nc.vector.memset(acc, 0.0)

        q3 = qbuf[:, :].rearrange("p (g s) -> p g s", s=64)
        a3 = lambda t_: t_[:, :].rearrange("p (g a) -> p g a", a=A)
        NQ = 8  # epilogue emitted in eighths
        HG = GR // NQ

        def ep_front(hh):
            # argmax/onehot of actions: independent of the MLP, runs early
            gsl = slice(hh * HG, (hh + 1) * HG)
            asl = slice(hh * HG * A, (hh + 1) * HG * A)
            iot_b = iota_s[:, None, :].broadcast_to([128, HG, A])
            if _ep():
                nc.scalar.activation(actf[:, asl], actb_s[:, asl],
                                     AF.Copy, scale=32.0)
            if _ep():
                nc.gpsimd.tensor_tensor(a3(score)[:, gsl], a3(actf)[:, gsl],
                                        iot_b, OP.subtract)
            if _ep():
                nc.vector.tensor_reduce(rowmax[:, gsl], a3(score)[:, gsl],
                                        AX.X, OP.max)
            if _ep():
                nc.vector.tensor_tensor(
                    a3(onehot)[:, gsl], a3(score)[:, gsl],
                    rowmax[:, gsl, None].broadcast_to([128, HG, A]),
                    OP.is_equal)
            if _ep():
                nc.vector.tensor_scalar(donem[:, gsl], s1b_s[:, gsl],
                                        DONE, None, OP.is_equal)
            if _ep():
                nc.vector.tensor_scalar(fac[:, gsl], donem[:, gsl],
                                        -DISC, DISC, OP.mult, OP.add)

        def ep_tail(hh):
            # needs qbuf for groups in the quarter
            gsl = slice(hh * HG, (hh + 1) * HG)
            if _ep():
                nc.gpsimd.tensor_tensor(a3(prod)[:, gsl], a3(onehot)[:, gsl],
                                        q3[:, gsl, 0:A], OP.mult)
            if _ep():
                nc.vector.tensor_reduce(q0sel[:, gsl], a3(prod)[:, gsl],
                                        AX.X, OP.add)
            if _ep():
                nc.vector.tensor_reduce(maxqn[:, gsl], q3[:, gsl, QOFF:QOFF + A],
                                        AX.X, OP.max)
            if _ep():
                nc.vector.tensor_tensor(t1[:, gsl], maxqn[:, gsl], fac[:, gsl],
                                        OP.mult)
            if _ep():
                nc.vector.tensor_tensor(t2[:, gsl], t1[:, gsl], rewb_s[:, gsl],
                                        OP.add)
            if _ep():
                nc.vector.tensor_tensor(diff[:, gsl], q0sel[:, gsl], t2[:, gsl],
                                        OP.subtract)
            if _ep():
                nc.vector.tensor_tensor(sq[:, gsl], diff[:, gsl], diff[:, gsl],
                                        OP.mult)

        def do_dma(li):
            x0L = xpool.tile([128, LOADCOLS], TILE_MDT, tag="x0")
            x1L = xpool.tile([128, LOADCOLS], TILE_MDT, tag="x1")
            nc.sync.dma_start(out=x0L,
                              in_=x0t[:, li * LOADCOLS:(li + 1) * LOADCOLS])
            nc.sync.dma_start(out=x1L,
                              in_=x1t[:, li * LOADCOLS:(li + 1) * LOADCOLS])
            xL_tiles[li] = (x0L, x1L)

        def do_dma_first():
            # split load 0 so tick 0/1 only wait on a small first piece
            xA = []
            for pa, src in ((0, x0t), (1, x1t)):
                t_ = xpool.tile([128, CH], TILE_MDT, tag=f"xA{pa}", bufs=1)
                nc.sync.dma_start(out=t_, in_=src[:, 0:CH])
                xA.append(t_)
            xB = []
            for pa, src in ((0, x0t), (1, x1t)):
                t_ = xpool.tile([128, LOADCOLS - CH], TILE_MDT, tag=f"xB{pa}",
                                bufs=1)
                nc.sync.dma_start(out=t_, in_=src[:, CH:LOADCOLS])
                xB.append(t_)
            xL_tiles[0] = (xA, xB)

        def xs_for(t):
            c, pa = t // 2, t % 2
            li = (c * CH) // LOADCOLS
            ci = (c * CH) % LOADCOLS // CH
            if li == 0:
                xA, xB = xL_tiles[0]
                if ci == 0:
                    return xA[pa][:, :]
                return xB[pa][:, (ci - 1) * CH:ci * CH]
            return xL_tiles[li][pa][:, ci * CH:(ci + 1) * CH]

        def st_mm1(t):
            h1p = ps_h1.tile([128, 2, CH], F32, tag="h1p", name=f"h1p_{t}")
            xs = xs_for(t)
            for m in range(2):
                nc.tensor.matmul(h1p[:, m, :], w1_s[m],
                                 xs, start=True, stop=True)
            h1p_t[t] = h1p

        def st_relu1(t):
            h1s = hpool.tile([128, 2, CH], H1_DT, tag="h1s", bufs=3,
                             name=f"h1s_{t}")
            relu_copy(h1s[:, :, :].rearrange("p a b -> p (a b)"),
                      h1p_t.pop(t)[:, :, :].rearrange("p a b -> p (a b)"),
                      b1_s[:, 0:1])
            h1s_t[t] = h1s

        DR_MODE = {"dr": mybir.MatmulPerfMode.DoubleRow,
                   "drsw": mybir.MatmulPerfMode.DoubleRowSwInterleave}.get(MM2)

        def st_mm2(t):
            h2p = ps_h2.tile([128, 2, CH], F32, tag="h2p", name=f"h2p_{t}")
            h1s = h1s_t.pop(t)
            if DR_MODE is not None:
                for m in range(2):
                    nc.tensor.matmul(h2p[:, m, :], w2_s[m][:, :, :],
                                     h1s[:, :, :], start=True, stop=True,
                                     perf_mode=DR_MODE)
            else:
                for m in range(2):
                    for k in range(2):
                        nc.tensor.matmul(h2p[:, m, :],
                                         w2_s[k][m],
                                         h1s[:, k, :], start=(k == 0), stop=(k == 1))
            h2p_t[t] = h2p

        def st_relu2(t):
            h2s = hpool.tile([128, 2, CH], TILE_MDT, tag="h2s", bufs=8,
                             name=f"h2s_{t}")
            relu_copy(h2s[:, :, :].rearrange("p a b -> p (a b)"),
                      h2p_t.pop(t)[:, :, :].rearrange("p a b -> p (a b)"),
                      b2_s[:, 0:1])
            h2s_t[t] = h2s

        def st_mm3(p):
            # pack 4 ticks into one PSUM tile via 32-wide col groups.
            # Complete each group's k-chain before starting the next group's
            # (a start=True clears has_written bits bank-wide).
            qt_ps = ps_qt.tile([128, CH], F32, tag="qt", name=f"qt_{p}")
            for g in range(4):
                h2s = h2s_t.pop(4 * p + g)
                for k in range(2):
                    nc.tensor.matmul(qt_ps[32 * g:32 * g + 32, :],
                                     w3_s[k], h2s[:, k, :],
                                     start=(k == 0), stop=(k == 1),
                                     tile_position=(0, 32 * g))
            qt_p[p] = qt_ps

        def st_stack(p):
            qts = qspool.tile([128, CH], TILE_MDT, tag="qts", name=f"qts_{p}")
            nc.scalar.activation(qts, qt_p.pop(p), AF.Identity, bias=b3_s,
                                 scale=1.0)
            qts_p[p] = qts

        def st_tp(p):
            tp_ps = ps_tp.tile([128, 2, 128], TILE_MDT, tag="tp", name=f"tp_{p}")
            qts = qts_p.pop(p)
            for j in range(2):
                nc.tensor.transpose(tp_ps[:, j, :],
                                    qts[:, j * 128:(j + 1) * 128], ident)
            tp_p[p] = tp_ps

        def st_qb(p):
            nc.vector.tensor_copy(
                qbuf[:, p * 2 * 128:(p + 1) * 2 * 128],
                tp_p.pop(p)[:, :, :].rearrange("p a b -> p (a b)"))

        do_dma_first()
        stage_rest()
        tails_done = 0
        for t in range(T + 11):
            # prefetch next x load 4 passes early
            nt = t + 4
            if nt < T and nt % PASS_PER_LOAD == 0:
                do_dma(nt // PASS_PER_LOAD)
            if t == 6:
                nc.sync.dma_start(out=actb_s, in_=actb)
                nc.sync.dma_start(out=rewb_s, in_=rewb)
                nc.sync.dma_start(out=s1b_s, in_=s1b)
            if t >= 10 and (t - 10) % 2 == 0 and (t - 10) // 2 < NQ:
                ep_front((t - 10) // 2)
            if t < T:
                st_mm1(t)
            if 0 <= t - 1 < T:
                st_relu1(t - 1)
            if 0 <= t - 2 < T:
                st_mm2(t - 2)
            if 0 <= t - 3 < T:
                st_relu2(t - 3)
            if t >= 7 and (t - 7) % 4 == 0 and (t - 7) // 4 < NPACK:
                st_mm3((t - 7) // 4)
            if t >= 8 and (t - 8) % 4 == 0 and (t - 8) // 4 < NPACK:
                st_stack((t - 8) // 4)
            if t >= 9 and (t - 9) % 4 == 0 and (t - 9) // 4 < NPACK:
                st_tp((t - 9) // 4)
            if t >= 10 and (t - 10) % 4 == 0 and (t - 10) // 4 < NPACK:
                p = (t - 10) // 4
                st_qb(p)
                while tails_done < NQ - 1 and p + 1 >= (tails_done + 1) * (NPACK // NQ):
                    ep_tail(tails_done)
                    tails_done += 1
        while tails_done < NQ:
            ep_tail(tails_done)
            tails_done += 1
        if _ep():
            nc.vector.tensor_reduce(acc, sq, AX.X, OP.add)
        nc.scalar.dma_start(out=outp, in_=acc)

    nc.compile()
    return nc


_CACHE = {}


def _get_program():
    if "nc" not in _CACHE:
        _CACHE["nc"] = _build_program()
    return _CACHE["nc"]


# slab permutation: qbuf slot L holds batch slab PERM[L] (see st_qb layout)
PERM = np.array([4 * (l // 4) + (0, 2, 1, 3)[l % 4] for l in range(GR)])


def _prep_in_maps(inputs):
    st0 = np.asarray(inputs["states0"], dtype=np.float32)
    st1 = np.asarray(inputs["states1"], dtype=np.float32)
    act = np.asarray(inputs["actions"], dtype=np.int32)
    rew = np.asarray(inputs["rewards"], dtype=np.float32)
    W1 = np.asarray(inputs["W1"], dtype=np.float32).astype(NP_MDT)
    if MM2 in ("dr", "drsw"):
        W2f = np.asarray(inputs["W2"], dtype=np.float32)
        # [ki, m, i, mm]: logical lhsT[ki, i, :] = W2[128*i + ki, m-half]
        w2l = W2f.reshape(2, 128, 2, 128).transpose(1, 2, 0, 3)  # [ki, m, i, mm]
        if MM2 == "drsw":
            # interleaved + column-reversed per sim: buf[p, 2*j+i] = logical[p, i, 127-j]
            rev = w2l[:, :, :, ::-1]                     # [ki, m, i, j]
            swi = rev.transpose(0, 1, 3, 2).reshape(128, 2, 128 * 2)  # [ki, m, (j i)]
            W2 = np.ascontiguousarray(swi.reshape(128, 2, 2, 128)).astype(NP_FP8)
        else:
            W2 = np.ascontiguousarray(w2l).astype(NP_FP8)
    else:
        W2 = np.asarray(inputs["W2"], dtype=np.float32).astype(NP_MDT)
    W3 = np.asarray(inputs["W3"], dtype=np.float32)
    b1 = np.asarray(inputs["b1"], dtype=np.float32)
    b2 = np.asarray(inputs["b2"], dtype=np.float32)
    b3 = np.asarray(inputs["b3"], dtype=np.float32)

    w3pad = np.zeros((H, QW), np.float32)
    w3pad[:, :A] = W3
    w3pad = w3pad.astype(NP_MDT)
    b1m = np.ascontiguousarray(b1.reshape(2, 128).T)
    b2m = np.ascontiguousarray(b2.reshape(2, 128).T)
    b3p = np.zeros((128, 1), np.float32)
    b3p[0:A, 0] = b3
    b3p[QOFF:QOFF + A, 0] = b3
    b3p[64:64 + A, 0] = b3
    b3p[64 + QOFF:64 + QOFF + A, 0] = b3
    iota = np.ascontiguousarray(
        np.broadcast_to(np.arange(A, dtype=np.float32), (128, A)))
    ident = np.eye(128, dtype=np.float32).astype(NP_MDT)

    in_maps = []
    for c in range(NCORES):
        r0, r1 = c * BC, (c + 1) * BC
        in_maps.append({
            "x0t": np.ascontiguousarray(st0[r0:r1].T).astype(NP_MDT),
            "x1t": np.ascontiguousarray(st1[r0:r1].T).astype(NP_MDT),
            "actb": np.ascontiguousarray(
                act[r0:r1].reshape(GR, 128, A)[PERM].transpose(1, 0, 2).reshape(128, GR * A)),
            "rewb": np.ascontiguousarray(rew[r0:r1].reshape(GR, 128)[PERM].T),
            "s1b": np.ascontiguousarray(st1[r0:r1, 0].reshape(GR, 128)[PERM].T),
            "w1": W1, "w2": W2, "w3p": w3pad,
            "b1d": b1m, "b2d": b2m, "b3d": b3p, "iotad": iota,
            "identd": ident,
        })
    return in_maps


def _run(inputs, trace=False):
    nc = _get_program()
    in_maps = _prep_in_maps(inputs)
    res = run_bass_kernel_spmd(nc, in_maps, core_ids=list(range(NCORES)),
                               trace=trace)
    total = 0.0
    for r in res.results:
        total += float(np.asarray(r["outp"], dtype=np.float64).sum())
    return np.array(np.float32(total)), res


def kernel(**inputs) -> np.ndarray:
    val, _ = _run(inputs, trace=False)
    return val


# revision 21
# speedup vs baseline: 1.0116x; 1.0116x over previous
"""Bass/Trainium2 kernel for nn_BellmanLoss (8-core data-parallel).

Math: the reference's scatter makes Q_new differ from Q0 only at
a_i = argmax_j(actions[i, j]) (first max), so

    loss = sum_i (Q0[i, a_i] - target_i)^2
    target_i = r_i + 0.9 * max_a Qn[i, a] * (1 - done_i),   done_i = (states1[i,0] == 666)

Per core: 8192 rows. MLP runs feature-major (h^T = [features, batch]) so the
weights are the stationary matmul operand. Layer 3 is packed 4 ticks at a
time into one PSUM tile via 32-wide col groups (tile_position), so the
narrow (A=18) output stops wasting full-width PE passes; the packed tile is
PE-transposed to batch-major in [128,128] blocks and a fused vector epilogue
computes the argmax-select, max_a, target and per-partition loss partials.
Host does layout-only prep (transpose/reshape/cast, group permutation) and
the final 1024-element sum.
"""

import os
import numpy as np

import concourse.bass as bass
import concourse.mybir as mybir
import concourse.tile as tile
from concourse import bacc
from concourse.bass_utils import run_bass_kernel_spmd

# Problem constants (hardcoded per contract)
B, S, H, A = 65536, 128, 256, 18
NCORES = 8
BC = B // NCORES          # 8192 rows per core
CH = 256                  # batch columns per compute chunk
NCH = BC // CH            # 32 chunks
GR = BC // 128            # 64 groups of 128 rows per core
LOADCOLS = 1024           # x DMA tile columns
NLOAD = BC // LOADCOLS
QW = 32                   # padded action dim per col group
QOFF = 32                 # Qn offset within a 64-col group slot
NPACK = 16                # mm3 packs (4 ticks each)
DONE = 666.0
DISC = 0.9

MM_DT = os.environ.get("BELLMAN_MM_DT", "bf16")  # "bf16" | "f32r"
MM2 = os.environ.get("BELLMAN_MM2", "bf16")  # "bf16" | "dr" | "drsw"
EP_LIMIT = int(os.environ.get("BELLMAN_EP", "9999"))

if MM_DT == "bf16":
    import ml_dtypes
    NP_MDT = ml_dtypes.bfloat16
    TILE_MDT = mybir.dt.bfloat16
else:
    NP_MDT = np.float32
    TILE_MDT = mybir.dt.float32r

F32 = mybir.dt.float32
I32 = mybir.dt.int32
FP8 = mybir.dt.float8e4
import ml_dtypes as _mld
NP_FP8 = _mld.float8_e4m3
H1_DT = FP8 if MM2 in ("dr", "drsw") else TILE_MDT
NP_H1 = NP_FP8 if MM2 in ("dr", "drsw") else NP_MDT
AF = mybir.ActivationFunctionType
OP = mybir.AluOpType
AX = mybir.AxisListType


def _build_program():
    nc = bacc.Bacc("TRN2", target_bir_lowering=False, debug=False)

    x0t = nc.dram_tensor("x0t", [128, BC], TILE_MDT, kind="ExternalInput").ap()
    x1t = nc.dram_tensor("x1t", [128, BC], TILE_MDT, kind="ExternalInput").ap()
    actb = nc.dram_tensor("actb", [128, GR * A], I32, kind="ExternalInput").ap()
    rewb = nc.dram_tensor("rewb", [128, GR], F32, kind="ExternalInput").ap()
    s1b = nc.dram_tensor("s1b", [128, GR], F32, kind="ExternalInput").ap()
    w1 = nc.dram_tensor("w1", [S, H], TILE_MDT, kind="ExternalInput").ap()
    if MM2 in ("dr", "drsw"):
        w2 = nc.dram_tensor("w2", [128, 2, 2, 128], FP8, kind="ExternalInput").ap()
    else:
        w2 = nc.dram_tensor("w2", [H, H], TILE_MDT, kind="ExternalInput").ap()
    w3p = nc.dram_tensor("w3p", [H, QW], TILE_MDT, kind="ExternalInput").ap()
    b1d = nc.dram_tensor("b1d", [128, 2], F32, kind="ExternalInput").ap()
    b2d = nc.dram_tensor("b2d", [128, 2], F32, kind="ExternalInput").ap()
    b3d = nc.dram_tensor("b3d", [128, 1], F32, kind="ExternalInput").ap()
    iotad = nc.dram_tensor("iotad", [128, A], F32, kind="ExternalInput").ap()
    identd = nc.dram_tensor("identd", [128, 128], TILE_MDT, kind="ExternalInput").ap()
    outp = nc.dram_tensor("outp", [128, 1], F32, kind="ExternalOutput").ap()

    from contextlib import ExitStack

    with tile.TileContext(nc) as tc, ExitStack() as ctx:
        singles = ctx.enter_context(tc.tile_pool(name="singles", bufs=1))
        xpool = ctx.enter_context(tc.tile_pool(name="xpool", bufs=2))
        hpool = ctx.enter_context(tc.tile_pool(name="hpool", bufs=2))
        qspool = ctx.enter_context(tc.tile_pool(name="qspool", bufs=2))
        big = ctx.enter_context(tc.tile_pool(name="big", bufs=1))
        ps_h1 = ctx.enter_context(tc.tile_pool(name="ps_h1", bufs=3, space="PSUM"))
        ps_h2 = ctx.enter_context(tc.tile_pool(name="ps_h2", bufs=2, space="PSUM"))
        ps_qt = ctx.enter_context(tc.tile_pool(name="ps_qt", bufs=1, space="PSUM"))
        ps_tp = ctx.enter_context(tc.tile_pool(name="ps_tp", bufs=2, space="PSUM"))

        # --- PE warmup: scratch matmuls during the initial DMA wait flip the
        # HAM clock gate to 8/8 before real matmuls arrive ---
        warm_w = singles.tile([128, 128], TILE_MDT, tag="warm_w")
        nc.gpsimd.memset(warm_w, 0.0)
        warm_act = singles.tile([128, 8], F32, tag="warm_act")
        nc.scalar.activation(warm_act, warm_w[:, 0:8], AF.Relu, scale=1.0)
        warm_ps = ps_qt.tile([128, CH], F32, tag="qt", name="warm_ps")
        for i in range(28):
            nc.tensor.matmul(warm_ps[:, 0:128], warm_w, warm_w,
                             start=True, stop=True)

        # --- constants / per-core staging loads (gpsimd queue: keeps the
        # scalar queue free so the first relu copies are not stuck behind
        # slow DMA-issue ops) ---
        w1_s = []
        for m in range(2):
            t = singles.tile([128, 128], TILE_MDT, tag=f"w1_{m}")
            nc.gpsimd.dma_start(out=t, in_=w1[:, m * 128:(m + 1) * 128])
            w1_s.append(t)
        b1_s = singles.tile([128, 2], F32, tag="b1")
        nc.gpsimd.dma_start(out=b1_s, in_=b1d)
        b2_s = singles.tile([128, 2], F32, tag="b2")
        nc.gpsimd.dma_start(out=b2_s, in_=b2d)
        if MM2 in ("dr", "drsw"):
            w2_s = []
            for m in range(2):
                t = singles.tile([128, 2, 128], FP8, tag=f"w2dr_{m}")
                nc.gpsimd.dma_start(out=t, in_=w2[:, m, :, :])
                w2_s.append(t)
        else:
            w2_s = []
            for k in range(2):
                row = []
                for m in range(2):
                    t = singles.tile([128, 128], TILE_MDT, tag=f"w2_{k}{m}")
                    nc.gpsimd.dma_start(
                        out=t, in_=w2[k * 128:(k + 1) * 128, m * 128:(m + 1) * 128])
                    row.append(t)
                w2_s.append(row)
        w3_s = []
        for k in range(2):
            t = singles.tile([128, QW], TILE_MDT, tag=f"w3_{k}", name=f"w3s_{k}")
            w3_s.append(t)
        b3_s = singles.tile([128, 1], F32, tag="b3")
        ident = singles.tile([128, 128], TILE_MDT, tag="ident")
        iota_s = singles.tile([128, A], F32, tag="iota")

        def stage_rest():
            # issued on sync after the first x pieces; none needed before ~17us
            for k in range(2):
                nc.sync.dma_start(out=w3_s[k], in_=w3p[k * 128:(k + 1) * 128, :])
            nc.sync.dma_start(out=b3_s, in_=b3d)
            nc.sync.dma_start(out=ident, in_=identd)
            nc.sync.dma_start(out=iota_s, in_=iotad)

        actb_s = singles.tile([128, GR * A], I32, tag="actb")
        rewb_s = singles.tile([128, GR], F32, tag="rewb")
        s1b_s = singles.tile([128, GR], F32, tag="s1b")

        # batch-major Q staging: slot L (one 128-row slab, host-permuted order)
        # occupies cols [64L, 64L+64): Q at +0..17 (states0), +32..49 (states1).
        qbuf = big.tile([128, GR * 64], TILE_MDT, tag="qbuf")

        relu_idx = [0]

        def relu_copy(dst, src, bias_ap):
            # split relu copies ~17:15 ACT:DVE to equalize engine time
            i = relu_idx[0] % 32
            relu_idx[0] += 1
            if i % 2 == 0 or i == 1:
                nc.scalar.activation(dst, src, AF.Relu, bias=bias_ap, scale=1.0)
            else:
                nc.vector.tensor_scalar(dst, src, bias_ap, 0.0, OP.add, OP.max)

        # ---- software-pipelined main loop ----
        # tick = one chunk-pass (64 ticks). Stage shifts keep every engine's
        # in-order queue free of waits on freshly produced cross-engine data:
        #   t:   mm1[t]          (PE)
        #   t+1: relu1[t]        (ACT/DVE)
        #   t+2: mm2[t]          (PE)
        #   t+3: relu2[t]        (ACT/DVE)
        #   pack p = ticks 4p..4p+3 (2 chunks x 2 states):
        #   4p+7: mm3 pack (PE, 4 col groups), 4p+8: stack copy,
        #   4p+9: transposes, 4p+10: qbuf copy
        T = 2 * NCH
        PASS_PER_LOAD = 2 * LOADCOLS // CH
        xL_tiles = {}
        h1p_t, h1s_t, h2p_t, h2s_t = {}, {}, {}, {}
        qt_p, qts_p, tp_p = {}, {}, {}

        # epilogue tiles (allocated up front; ops emitted inline)
        import itertools
        _ep_count = itertools.count(1)

        def _ep():
            return next(_ep_count) <= EP_LIMIT

        actf = big.tile([128, GR * A], F32, tag="actf")
        score = big.tile([128, GR * A], F32, tag="score")
        rowmax = big.tile([128, GR], F32, tag="rowmax")
        onehot = big.tile([128, GR * A], F32, tag="onehot")
        prod = big.tile([128, GR * A], F32, tag="prod")
        q0sel = big.tile([128, GR], F32, tag="q0sel")
        maxqn = big.tile([128, GR], F32, tag="maxqn")
        donem = big.tile([128, GR], F32, tag="donem")
        fac = big.tile([128, GR], F32, tag="fac")
        t1 = big.tile([128, GR], F32, tag="t1")
        t2 = big.tile([128, GR], F32, tag="t2")
        diff = big.tile([128, GR], F32, tag="diff")
        sq = big.tile([128, GR], F32, tag="sq")
        acc = big.tile([128, 1], F32, tag="acc")
        if EP_LIMIT < 9999:
            nc.vector.memset(acc, 0.0)

        q3 = qbuf[:, :].rearrange("p (g s) -> p g s", s=64)
        a3 = lambda t_: t_[:, :].rearrange("p (g a) -> p g a", a=A)
        NQ = 8  # epilogue emitted in eighths
        HG = GR // NQ

        def ep_front(hh):
            # argmax/onehot of actions: independent of the MLP, runs early
            gsl = slice(hh * HG, (hh + 1) * HG)
            asl = slice(hh * HG * A, (hh + 1) * HG * A)
            iot_b = iota_s[:, None, :].broadcast_to([128, HG, A])
            if _ep():
                nc.scalar.activation(actf[:, asl], actb_s[:, asl],
                                     AF.Copy, scale=32.0)
            if _ep():
                nc.gpsimd.tensor_tensor(a3(score)[:, gsl], a3(actf)[:, gsl],
                                        iot_b, OP.subtract)
            if _ep():
                nc.vector.tensor_reduce(rowmax[:, gsl], a3(score)[:, gsl],
                                        AX.X, OP.max)
            if _ep():
                nc.vector.tensor_tensor(
                    a3(onehot)[:, gsl], a3(score)[:, gsl],
                    rowmax[:, gsl, None].broadcast_to([128, HG, A]),
                    OP.is_equal)
            if _ep():
                nc.vector.tensor_scalar(donem[:, gsl], s1b_s[:, gsl],
                                        DONE, None, OP.is_equal)
            if _ep():
                nc.vector.tensor_scalar(fac[:, gsl], donem[:, gsl],
                                        -DISC, DISC, OP.mult, OP.add)

        def ep_tail(hh):
            # needs qbuf for groups in the quarter
            gsl = slice(hh * HG, (hh + 1) * HG)
            if _ep():
                nc.gpsimd.tensor_tensor(a3(prod)[:, gsl], a3(onehot)[:, gsl],
                                        q3[:, gsl, 0:A], OP.mult)
            if _ep():
                nc.vector.tensor_reduce(q0sel[:, gsl], a3(prod)[:, gsl],
                                        AX.X, OP.add)
            if _ep():
                nc.vector.tensor_reduce(maxqn[:, gsl], q3[:, gsl, QOFF:QOFF + A],
                                        AX.X, OP.max)
            if _ep():
                nc.vector.tensor_tensor(t1[:, gsl], maxqn[:, gsl], fac[:, gsl],
                                        OP.mult)
            if _ep():
                nc.vector.tensor_tensor(t2[:, gsl], t1[:, gsl], rewb_s[:, gsl],
                                        OP.add)
            if _ep():
                nc.vector.tensor_tensor(diff[:, gsl], q0sel[:, gsl], t2[:, gsl],
                                        OP.subtract)
            if _ep():
                nc.vector.tensor_tensor(sq[:, gsl], diff[:, gsl], diff[:, gsl],
                                        OP.mult)

        def do_dma(li):
            x0L = xpool.tile([128, LOADCOLS], TILE_MDT, tag="x0")
            x1L = xpool.tile([128, LOADCOLS], TILE_MDT, tag="x1")
            nc.sync.dma_start(out=x0L,
                              in_=x0t[:, li * LOADCOLS:(li + 1) * LOADCOLS])
            nc.sync.dma_start(out=x1L,
                              in_=x1t[:, li * LOADCOLS:(li + 1) * LOADCOLS])
            xL_tiles[li] = (x0L, x1L)

        def do_dma_first():
            # split load 0 so tick 0/1 only wait on a small first piece
            xA = []
            for pa, src in ((0, x0t), (1, x1t)):
                t_ = xpool.tile([128, CH], TILE_MDT, tag=f"xA{pa}", bufs=1)
                nc.sync.dma_start(out=t_, in_=src[:, 0:CH])
                xA.append(t_)
            xB = []
            for pa, src in ((0, x0t), (1, x1t)):
                t_ = xpool.tile([128, LOADCOLS - CH], TILE_MDT, tag=f"xB{pa}",
                                bufs=1)
                nc.sync.dma_start(out=t_, in_=src[:, CH:LOADCOLS])
                xB.append(t_)
            xL_tiles[0] = (xA, xB)

        def xs_for(t):
            c, pa = t // 2, t % 2
            li = (c * CH) // LOADCOLS
            ci = (c * CH) % LOADCOLS // CH
            if li == 0:
                xA, xB = xL_tiles[0]
                if ci == 0:
                    return xA[pa][:, :]
                return xB[pa][:, (ci - 1) * CH:ci * CH]
            return xL_tiles[li][pa][:, ci * CH:(ci + 1) * CH]

        def st_mm1(t):
            h1p = ps_h1.tile([128, 2, CH], F32, tag="h1p", name=f"h1p_{t}")
            xs = xs_for(t)
            for m in range(2):
                nc.tensor.matmul(h1p[:, m, :], w1_s[m],
                                 xs, start=True, stop=True)
            h1p_t[t] = h1p

        def st_relu1(t):
            h1s = hpool.tile([128, 2, CH], H1_DT, tag="h1s", bufs=3,
                             name=f"h1s_{t}")
            relu_copy(h1s[:, :, :].rearrange("p a b -> p (a b)"),
                      h1p_t.pop(t)[:, :, :].rearrange("p a b -> p (a b)"),
                      b1_s[:, 0:1])
            h1s_t[t] = h1s

        DR_MODE = {"dr": mybir.MatmulPerfMode.DoubleRow,
                   "drsw": mybir.MatmulPerfMode.DoubleRowSwInterleave}.get(MM2)

        def st_mm2(t):
            h2p = ps_h2.tile([128, 2, CH], F32, tag="h2p", name=f"h2p_{t}")
            h1s = h1s_t.pop(t)
            if DR_MODE is not None:
                for m in range(2):
                    nc.tensor.matmul(h2p[:, m, :], w2_s[m][:, :, :],
                                     h1s[:, :, :], start=True, stop=True,
                                     perf_mode=DR_MODE)
            else:
                for m in range(2):
                    for k in range(2):
                        nc.tensor.matmul(h2p[:, m, :],
                                         w2_s[k][m],
                                         h1s[:, k, :], start=(k == 0), stop=(k == 1))
            h2p_t[t] = h2p

        def st_relu2(t):
            h2s = hpool.tile([128, 2, CH], TILE_MDT, tag="h2s", bufs=8,
                             name=f"h2s_{t}")
            relu_copy(h2s[:, :, :].rearrange("p a b -> p (a b)"),
                      h2p_t.pop(t)[:, :, :].rearrange("p a b -> p (a b)"),
                      b2_s[:, 0:1])
            h2s_t[t] = h2s

        def st_mm3(p):
            # pack 4 ticks into one PSUM tile via 32-wide col groups.
            # Complete each group's k-chain before starting the next group's
            # (a start=True clears has_written bits bank-wide).
            qt_ps = ps_qt.tile([128, CH], F32, tag="qt", name=f"qt_{p}")
            for g in range(4):
                h2s = h2s_t.pop(4 * p + g)
                for k in range(2):
                    nc.tensor.matmul(qt_ps[32 * g:32 * g + 32, :],
                                     w3_s[k], h2s[:, k, :],
                                     start=(k == 0), stop=(k == 1),
                                     tile_position=(0, 32 * g))
            qt_p[p] = qt_ps

        def st_stack(p):
            qts = qspool.tile([128, CH], TILE_MDT, tag="qts", name=f"qts_{p}")
            nc.scalar.activation(qts, qt_p.pop(p), AF.Identity, bias=b3_s,
                                 scale=1.0)
            qts_p[p] = qts

        def st_tp(p):
            tp_ps = ps_tp.tile([128, 2, 128], TILE_MDT, tag="tp", name=f"tp_{p}")
            qts = qts_p.pop(p)
            for j in range(2):
                nc.tensor.transpose(tp_ps[:, j, :],
                                    qts[:, j * 128:(j + 1) * 128], ident)
            tp_p[p] = tp_ps

        def st_qb(p):
            nc.vector.tensor_copy(
                qbuf[:, p * 2 * 128:(p + 1) * 2 * 128],
                tp_p.pop(p)[:, :, :].rearrange("p a b -> p (a b)"))

        do_dma_first()
        stage_rest()
        tails_done = 0
        for t in range(T + 11):
            # prefetch next x load 4 passes early
            nt = t + 4
            if nt < T and nt % PASS_PER_LOAD == 0:
                do_dma(nt // PASS_PER_LOAD)
            if t == 6:
                nc.sync.dma_start(out=actb_s, in_=actb)
                nc.sync.dma_start(out=rewb_s, in_=rewb)
                nc.sync.dma_start(out=s1b_s, in_=s1b)
            if t >= 10 and (t - 10) % 2 == 0 and (t - 10) // 2 < NQ:
                ep_front((t - 10) // 2)
            if t < T:
                st_mm1(t)
            if 0 <= t - 1 < T:
                st_relu1(t - 1)
            if 0 <= t - 2 < T:
                st_mm2(t - 2)
            if 0 <= t - 3 < T:
                st_relu2(t - 3)
            if t >= 7 and (t - 7) % 4 == 0 and (t - 7) // 4 < NPACK:
                st_mm3((t - 7) // 4)
            if t >= 8 and (t - 8) % 4 == 0 and (t - 8) // 4 < NPACK:
                st_stack((t - 8) // 4)
            if t >= 9 and (t - 9) % 4 == 0 and (t - 9) // 4 < NPACK:
                st_tp((t - 9) // 4)
            if t >= 10 and (t - 10) % 4 == 0 and (t - 10) // 4 < NPACK:
                p = (t - 10) // 4
                st_qb(p)
                while tails_done < NQ - 1 and p + 1 >= (tails_done + 1) * (NPACK // NQ):
                    ep_tail(tails_done)
                    tails_done += 1
        while tails_done < NQ:
            ep_tail(tails_done)
            tails_done += 1
        if _ep():
            nc.vector.tensor_reduce(acc, sq, AX.X, OP.add)
        nc.scalar.dma_start(out=outp, in_=acc)

    nc.compile()
    return nc


_CACHE = {}


def _get_program():
    if "nc" not in _CACHE:
        _CACHE["nc"] = _build_program()
    return _CACHE["nc"]


# slab permutation: qbuf slot L holds batch slab PERM[L] (see st_qb layout)
PERM = np.array([4 * (l // 4) + (0, 2, 1, 3)[l % 4] for l in range(GR)])


def _prep_in_maps(inputs):
    st0 = np.asarray(inputs["states0"], dtype=np.float32)
    st1 = np.asarray(inputs["states1"], dtype=np.float32)
    act = np.asarray(inputs["actions"], dtype=np.int32)
    rew = np.asarray(inputs["rewards"], dtype=np.float32)
    W1 = np.asarray(inputs["W1"], dtype=np.float32).astype(NP_MDT)
    if MM2 in ("dr", "drsw"):
        W2f = np.asarray(inputs["W2"], dtype=np.float32)
        # [ki, m, i, mm]: logical lhsT[ki, i, :] = W2[128*i + ki, m-half]
        w2l = W2f.reshape(2, 128, 2, 128).transpose(1, 2, 0, 3)  # [ki, m, i, mm]
        if MM2 == "drsw":
            # interleaved + column-reversed per sim: buf[p, 2*j+i] = logical[p, i, 127-j]
            rev = w2l[:, :, :, ::-1]                     # [ki, m, i, j]
            swi = rev.transpose(0, 1, 3, 2).reshape(128, 2, 128 * 2)  # [ki, m, (j i)]
            W2 = np.ascontiguousarray(swi.reshape(128, 2, 2, 128)).astype(NP_FP8)
        else:
            W2 = np.ascontiguousarray(w2l).astype(NP_FP8)
    else:
        W2 = np.asarray(inputs["W2"], dtype=np.float32).astype(NP_MDT)
    W3 = np.asarray(inputs["W3"], dtype=np.float32)
    b1 = np.asarray(inputs["b1"], dtype=np.float32)
    b2 = np.asarray(inputs["b2"], dtype=np.float32)
    b3 = np.asarray(inputs["b3"], dtype=np.float32)

    w3pad = np.zeros((H, QW), np.float32)
    w3pad[:, :A] = W3
    w3pad = w3pad.astype(NP_MDT)
    b1m = np.ascontiguousarray(b1.reshape(2, 128).T)
    b2m = np.ascontiguousarray(b2.reshape(2, 128).T)
    b3p = np.zeros((128, 1), np.float32)
    b3p[0:A, 0] = b3
    b3p[QOFF:QOFF + A, 0] = b3
    b3p[64:64 + A, 0] = b3
    b3p[64 + QOFF:64 + QOFF + A, 0] = b3
    iota = np.ascontiguousarray(
        np.broadcast_to(np.arange(A, dtype=np.float32), (128, A)))
    ident = np.eye(128, dtype=np.float32).astype(NP_MDT)

    in_maps = []
    for c in range(NCORES):
        r0, r1 = c * BC, (c + 1) * BC
        in_maps.append({
            "x0t": np.ascontiguousarray(st0[r0:r1].T).astype(NP_MDT),
            "x1t": np.ascontiguousarray(st1[r0:r1].T).astype(NP_MDT),
            "actb": np.ascontiguousarray(
                act[r0:r1].reshape(GR, 128, A)[PERM].transpose(1, 0, 2).reshape(128, GR * A)),
            "rewb": np.ascontiguousarray(rew[r0:r1].reshape(GR, 128)[PERM].T),
            "s1b": np.ascontiguousarray(st1[r0:r1, 0].reshape(GR, 128)[PERM].T),
            "w1": W1, "w2": W2, "w3p": w3pad,
            "b1d": b1m, "b2d": b2m, "b3d": b3p, "iotad": iota,
            "identd": ident,
        })
    return in_maps


def _run(inputs, trace=False):
    nc = _get_program()
    in_maps = _prep_in_maps(inputs)
    res = run_bass_kernel_spmd(nc, in_maps, core_ids=list(range(NCORES)),
                               trace=trace)
    total = 0.0
    for r in res.results:
        total += float(np.asarray(r["outp"], dtype=np.float64).sum())
    return np.array(np.float32(total)), res


def kernel(**inputs) -> np.ndarray:
    val, _ = _run(inputs, trace=False)
    return val
